# revision 9
# baseline (speedup 1.0000x reference)
"""Cascaded codebook embedding lookup on 8 trn2 NeuronCores.

Data-parallel: the 262144-token batch is sharded across 8 cores (32768
tokens each); the tiny 256x512 fp32 table (tiers concatenated) is
replicated to every core and lives in SBUF.

Per-core algorithm (one-hot matmul; bitexact, verified on HW):
  - The table is split on-device into float32r hi + float32r residual
    (f32r rounds fp32 to ~13 mantissa bits; hi + res reconstructs fp32
    bitexactly, and each f32r matmul streams at full PE rate, unlike
    plain fp32 which is 4x slower).
  - For each chunk of 512 tokens: broadcast-DMA the fp32 token ids to all
    128 partitions, build two [128, 512] one-hot-transposed tiles with
    is_equal against a [128,1] iota (k half 0: ids 0..127, half 1:
    128..255), then for each 128-row embed slice accumulate 4 matmuls in
    PSUM: hi0/res0 x oh0 + hi1/res1 x oh1. PSUM -> SBUF copies alternate
    between ScalarE and VectorE; stores batch 4 chunks into 1 MB DMAs.
  - Output is produced transposed ([512, 32768] per core, embed dim on
    partitions so the table is the stationary matmul operand); the host
    transposes back.
  - Invalid ids (outside [0, 256)) match no iota value -> all-zero
    one-hot column -> zero output row, matching the reference.
"""

from contextlib import ExitStack

import numpy as np

import concourse.bacc as bacc
import concourse.mybir as mybir
import concourse.tile as tile
from concourse.bass_utils import run_bass_kernel_spmd

N_CORES = 8
BATCH = 262144
B_LOC = BATCH // N_CORES  # 32768
D = 512
TOTAL = 256
CHUNK = 512  # tokens per psum tile (one full PSUM bank of fp32)
STORE_CHUNKS = 4  # chunks batched per output DMA (1 MB each)

f32 = mybir.dt.float32
f32r = mybir.dt.float32r


def _build_table_split(nc, tc, setup, tab, iota):
    """Load table + iota; produce f32r hi/residual tiles (one-time setup)."""
    t_raw = [setup.tile([128, D], f32, tag=f"traw{h}", name=f"traw{h}") for h in range(2)]
    hi = [setup.tile([128, D], f32r, tag=f"hi{h}", name=f"hi{h}") for h in range(2)]
    re = [setup.tile([128, D], f32r, tag=f"re{h}", name=f"re{h}") for h in range(2)]
    io = setup.tile([128, 2], f32)
    nc.sync.dma_start(io[:], iota[:])
    for h in range(2):
        nc.sync.dma_start(t_raw[h][:], tab[h])
        nc.vector.tensor_copy(hi[h][:], t_raw[h][:])
        nc.vector.tensor_tensor(
            out=re[h][:],
            in0=t_raw[h][:],
            in1=hi[h][:].bitcast(f32),
            op=mybir.AluOpType.subtract,
        )
    return hi, re, io


def _build_body(nc, tc, sb, obp, ps, hi, re, io, idxf, outt, n_chunks):
    """One full pass over n_chunks chunks of CHUNK tokens."""
    obufs = None
    for c in range(n_chunks):
        cs = slice(c * CHUNK, (c + 1) * CHUNK)
        idxr = sb.tile([128, CHUNK], f32, tag="idxr", name="idxr")
        nc.sync.dma_start(idxr[:], idxf[0:1, cs].to_broadcast([128, CHUNK]))
        oh = []
        for h in range(2):
            o = sb.tile([128, CHUNK], f32r, tag=f"oh{h}", name=f"oh{h}")
            nc.vector.tensor_tensor(
                out=o[:],
                in0=idxr[:],
                in1=io[:, h : h + 1].to_broadcast([128, CHUNK]),
                op=mybir.AluOpType.is_equal,
            )
            oh.append(o)
        if c % STORE_CHUNKS == 0:
            obufs = [
                obp.tile([128, STORE_CHUNKS * CHUNK], f32, tag=f"ob{d}", name=f"ob{d}")
                for d in range(4)
            ]
        off = (c % STORE_CHUNKS) * CHUNK
        for dsl in range(4):
            sl = slice(dsl * 128, (dsl + 1) * 128)
            psum = ps.tile([128, CHUNK], f32, space="PSUM", tag="psum", name="psum")
            nc.tensor.matmul(
                psum[:], lhsT=hi[0][:, sl], rhs=oh[0][:], start=True, stop=False
            )
            nc.tensor.matmul(
                psum[:], lhsT=re[0][:, sl], rhs=oh[0][:], start=False, stop=False
            )
            nc.tensor.matmul(
                psum[:], lhsT=hi[1][:, sl], rhs=oh[1][:], start=False, stop=False
            )
            nc.tensor.matmul(
                psum[:], lhsT=re[1][:, sl], rhs=oh[1][:], start=False, stop=True
            )
            dst = obufs[dsl][:, off : off + CHUNK]
            if dsl % 2 == 0:
                nc.scalar.copy(dst, psum[:])
            else:
                nc.vector.tensor_copy(dst, psum[:])
        if c % STORE_CHUNKS == STORE_CHUNKS - 1:
            gs = slice((c + 1 - STORE_CHUNKS) * CHUNK, (c + 1) * CHUNK)
            for dsl in range(4):
                nc.sync.dma_start(outt[dsl * 128 : (dsl + 1) * 128, gs], obufs[dsl][:])


def _build_nc(b_loc: int):
    n_chunks = b_loc // CHUNK
    nc = bacc.Bacc()
    tab = nc.declare_dram_parameter("table", [2, 128, D], f32, isOutput=False)
    idxf = nc.declare_dram_parameter("idxf", [1, b_loc], f32, isOutput=False)
    iota = nc.declare_dram_parameter("iota", [128, 2], f32, isOutput=False)
    outt = nc.declare_dram_parameter("outt", [D, b_loc], f32, isOutput=True)

    with tile.TileContext(nc) as tc, ExitStack() as ctx:
        setup = ctx.enter_context(tc.tile_pool(name="setup", bufs=1))
        sb = ctx.enter_context(tc.tile_pool(name="sb", bufs=2))
        obp = ctx.enter_context(tc.tile_pool(name="obp", bufs=2))
        ps = ctx.enter_context(tc.tile_pool(name="ps", bufs=8, space="PSUM"))
        hi, re, io = _build_table_split(nc, tc, setup, tab, iota)
        _build_body(nc, tc, sb, obp, ps, hi, re, io, idxf, outt, n_chunks)
    nc.compile()
    return nc


def _build_timing_nc(b_loc: int, loop_n: int):
    """Timing-only variant: same per-pass body, run loop_n times via a
    hardware loop; outt is internal DRAM and only a tiny dummy output is
    returned, so device->host transfer is negligible."""
    n_chunks = b_loc // CHUNK
    nc = bacc.Bacc()
    tab = nc.declare_dram_parameter("table", [2, 128, D], f32, isOutput=False)
    idxf = nc.declare_dram_parameter("idxf", [1, b_loc], f32, isOutput=False)
    iota = nc.declare_dram_parameter("iota", [128, 2], f32, isOutput=False)
    outt = nc.dram_tensor("outt_internal", [D, b_loc], f32)
    done = nc.declare_dram_parameter("done", [1, 2], f32, isOutput=True)

    with tile.TileContext(nc) as tc, ExitStack() as ctx:
        setup = ctx.enter_context(tc.tile_pool(name="setup", bufs=1))
        sb = ctx.enter_context(tc.tile_pool(name="sb", bufs=2))
        obp = ctx.enter_context(tc.tile_pool(name="obp", bufs=2))
        ps = ctx.enter_context(tc.tile_pool(name="ps", bufs=8, space="PSUM"))
        hi, re, io = _build_table_split(nc, tc, setup, tab, iota)
        with tc.For_i(0, loop_n, 1):
            _build_body(nc, tc, sb, obp, ps, hi, re, io, idxf, outt[:, :], n_chunks)
        nc.sync.dma_start(done[:], io[0:1, 0:2])
    nc.compile()
    return nc


_CACHE: dict = {}


def _get_nc(key, builder, *args):
    if key not in _CACHE:
        _CACHE[key] = builder(*args)
    return _CACHE[key]


def _iota_np():
    return np.stack(
        [np.arange(128, dtype=np.float32), np.arange(128, 256, dtype=np.float32)],
        axis=1,
    )


def _prep(indices, tier0, tier1, tier2):
    idx = np.asarray(indices).astype(np.int64).ravel()
    assert idx.shape[0] == BATCH, idx.shape
    valid = (idx >= 0) & (idx < TOTAL)
    idxf = np.where(valid, idx, -1).astype(np.float32)
    table = np.concatenate(
        [
            np.asarray(tier0, np.float32),
            np.asarray(tier1, np.float32),
            np.asarray(tier2, np.float32),
        ],
        axis=0,
    ).reshape(2, 128, D)
    iota = _iota_np()
    in_maps = [
        {
            "table": table,
            "iota": iota,
            "idxf": idxf[i * B_LOC : (i + 1) * B_LOC][None, :],
        }
        for i in range(N_CORES)
    ]
    return in_maps


def kernel(indices, tier0, tier1, tier2):
    in_maps = _prep(indices, tier0, tier1, tier2)
    nc = _get_nc(("mm", B_LOC), _build_nc, B_LOC)
    res = run_bass_kernel_spmd(nc, in_maps, list(range(N_CORES)))
    out = np.empty((BATCH, D), np.float32)
    for i in range(N_CORES):
        out[i * B_LOC : (i + 1) * B_LOC] = res.results[i]["outt"].T
    return out


def time_hw(inputs, loop_a: int = 4, loop_b: int = 54, n_runs: int = 6) -> float:
    """Estimate one full-pass HW time in ns by differencing two hardware-loop
    counts (axon/PJRT overhead and transfers cancel)."""
    import time

    in_maps = _prep(**inputs)
    ncA = _get_nc(("timing", B_LOC, loop_a), _build_timing_nc, B_LOC, loop_a)
    ncB = _get_nc(("timing", B_LOC, loop_b), _build_timing_nc, B_LOC, loop_b)
    cores = list(range(N_CORES))

    def best(nc):
        ts = []
        for _ in range(n_runs):
            t0 = time.time()
            run_bass_kernel_spmd(nc, in_maps, cores)
            ts.append(time.time() - t0)
        return min(ts)

    best(ncA)
    best(ncB)
    tA, tB = best(ncA), best(ncB)
    return (tB - tA) / (loop_b - loop_a) * 1e9
